# revision 1
# baseline (speedup 1.0000x reference)
"""JacobiKANLinear TRN2 Bass kernel.

out = silu(x) @ W_base^T + einsum('bik,oik->bo', P(tanh(x)), C) + bias

Host-side algebra: Jacobi polynomials (A=B=1, degree 5) are re-expressed in
the monomial basis.  D[o,i,j] = sum_k C[o,i,k] * T[k,j] where T holds the
monomial coefficients of P_k.  The j=0 term is constant (t^0 == 1) and folds
into the bias.  The device then computes 6 feature blocks
[silu(x), t, t^2, t^3, t^4, t^5] (t = tanh(x)) and one fused matmul with
contraction 6*1024 = 6144, plus a K=1 ones-x-bias matmul that initialises
PSUM with the bias broadcast.

Sharding (8 cores): 4 batch groups x 2 out-feature halves.  Per core:
batch shard 2048 rows, out shard 512 cols.  Matmuls run in float32r
(1 cycle/row on the PE at N=512 — bf16 speed with ~15x better accuracy).
"""
import numpy as np

import concourse.bass as bass
import concourse.mybir as mybir
import concourse.tile as tile
from concourse import bacc
from concourse.bass_utils import run_bass_kernel_spmd

BATCH = 8192
IN_F = 1024
OUT_F = 1024
DEGREE = 5
A = 1.0
B = 1.0

N_CORES = 8
BATCH_GROUPS = 4
OUT_HALVES = 2
B_SHARD = BATCH // BATCH_GROUPS        # 2048
O_SHARD = OUT_F // OUT_HALVES          # 512
N_BLOCKS = DEGREE + 1                  # 6 feature blocks
N_KT = N_BLOCKS * IN_F // 128          # 48 contraction tiles of 128
N_CHUNKS = B_SHARD // 128              # 16 batch chunks per core
IT_PER_BLOCK = IN_F // 128             # 8 in-feature tiles per block

F32 = mybir.dt.float32
F32R = mybir.dt.float32r


def _jacobi_monomial_matrix():
    """T[k, j] = coefficient of t^j in P_k (A=B=1), float64."""
    T = np.zeros((DEGREE + 1, DEGREE + 1), dtype=np.float64)
    polys = [np.zeros(DEGREE + 1) for _ in range(DEGREE + 1)]
    polys[0][0] = 1.0
    if DEGREE >= 1:
        # 0.5 * (2(A+1) t + (A-B))
        polys[1][1] = A + 1.0
        polys[1][0] = 0.5 * (A - B)
    for k in range(2, DEGREE + 1):
        alpha_n = 2.0 * k * (k + A + B) * (2 * k + A + B - 2)
        beta_n = (2 * k + A + B - 1) * (A ** 2 - B ** 2)
        gamma_n = (2 * k + A + B - 2) * (2 * k + A + B - 1) * (2 * k + A + B)
        delta_n = 2.0 * (k + A - 1) * (k + B - 1) * (2 * k + A + B)
        # P_k = ((beta + alpha t)/gamma) P_{k-1} - (delta/gamma) P_{k-2}
        p = np.zeros(DEGREE + 1)
        p += (beta_n / gamma_n) * polys[k - 1]
        p[1:] += (alpha_n / gamma_n) * polys[k - 1][:-1]
        p -= (delta_n / gamma_n) * polys[k - 2]
        polys[k] = p
    for k in range(DEGREE + 1):
        T[k] = polys[k]
    return T


def _build_nc():
    nc = bacc.Bacc()
    xt_in = nc.declare_dram_parameter(
        "xt", [IT_PER_BLOCK, 128, B_SHARD], F32, isOutput=False)
    w_in = nc.declare_dram_parameter(
        "w", [128, N_KT, O_SHARD], F32R, isOutput=False)
    bias_in = nc.declare_dram_parameter("biasv", [1, O_SHARD], F32R, isOutput=False)
    ones_in = nc.declare_dram_parameter("onesv", [1, 128], F32R, isOutput=False)
    out = nc.declare_dram_parameter("out", [B_SHARD, O_SHARD], F32, isOutput=True)

    with tile.TileContext(nc) as tc:
        with tc.tile_pool(name="wpool", bufs=1) as wpool, \
             tc.tile_pool(name="xpool", bufs=3) as xpool, \
             tc.tile_pool(name="fpool", bufs=2) as fpool, \
             tc.tile_pool(name="opool", bufs=3) as opool, \
             tc.tile_pool(name="psum", bufs=4, space="PSUM") as psum_pool:
            # Resident weights: one DMA per contraction tile so chunk-0
            # matmuls can start as slices land.
            w_sb = wpool.tile([128, N_KT, O_SHARD], F32R)
            for kt in range(N_KT):
                nc.sync.dma_start(out=w_sb[:, kt, :], in_=w_in[:, kt, :])
            bias_sb = wpool.tile([1, O_SHARD], F32R)
            nc.sync.dma_start(out=bias_sb[:], in_=bias_in[:])
            ones_sb = wpool.tile([1, 128], F32R)
            nc.sync.dma_start(out=ones_sb[:], in_=ones_in[:])

            for m in range(N_CHUNKS):
                bsl = bass.ts(m, 128)
                x_m = xpool.tile([128, IT_PER_BLOCK, 128], F32, tag="x")
                nc.sync.dma_start(
                    out=x_m[:], in_=xt_in[:, :, bsl].transpose([1, 0, 2]))

                silu_m = fpool.tile([128, IT_PER_BLOCK, 128], F32R, tag="silu")
                t_m = fpool.tile([128, IT_PER_BLOCK, 128], F32R, tag="t1")
                nc.scalar.activation(
                    silu_m[:], x_m[:], mybir.ActivationFunctionType.Silu)
                nc.scalar.activation(
                    t_m[:], x_m[:], mybir.ActivationFunctionType.Tanh)
                t2_m = fpool.tile([128, IT_PER_BLOCK, 128], F32R, tag="t2")
                nc.vector.tensor_mul(t2_m[:], t_m[:], t_m[:])
                t3_m = fpool.tile([128, IT_PER_BLOCK, 128], F32R, tag="t3")
                nc.vector.tensor_mul(t3_m[:], t2_m[:], t_m[:])
                t4_m = fpool.tile([128, IT_PER_BLOCK, 128], F32R, tag="t4")
                nc.vector.tensor_mul(t4_m[:], t3_m[:], t_m[:])
                t5_m = fpool.tile([128, IT_PER_BLOCK, 128], F32R, tag="t5")
                nc.vector.tensor_mul(t5_m[:], t4_m[:], t_m[:])
                blocks = [silu_m, t_m, t2_m, t3_m, t4_m, t5_m]

                ps = psum_pool.tile([128, O_SHARD], F32, tag="ps")
                # PSUM init: every row gets the bias vector (ones.T @ bias).
                nc.tensor.matmul(
                    ps[:], ones_sb[:], bias_sb[:], start=True, stop=False)
                for b in range(N_BLOCKS):
                    for it in range(IT_PER_BLOCK):
                        kt = b * IT_PER_BLOCK + it
                        nc.tensor.matmul(
                            ps[:], blocks[b][:, it, :], w_sb[:, kt, :],
                            start=False, stop=(kt == N_KT - 1))
                o_m = opool.tile([128, O_SHARD], F32, tag="o")
                nc.vector.tensor_copy(o_m[:], ps[:])
                nc.sync.dma_start(out=out[bsl, :], in_=o_m[:])
    nc.finalize()
    return nc


_NC_CACHE = None


def _get_nc():
    global _NC_CACHE
    if _NC_CACHE is None:
        _NC_CACHE = _build_nc()
    return _NC_CACHE


def _prepare_host(x, base_weight, jacobi_coeffs, bias):
    T = _jacobi_monomial_matrix()
    D = np.einsum("oik,kj->oij", jacobi_coeffs.astype(np.float64), T)
    bias_eff = bias.astype(np.float64) + D[:, :, 0].sum(axis=1)

    # W'[f, o]: 6 blocks of IN_F feature rows: silu -> base_weight, t^j -> D_j
    w_full = np.empty((N_BLOCKS * IN_F, OUT_F), dtype=np.float32)
    w_full[0:IN_F] = base_weight.T
    for j in range(1, N_BLOCKS):
        w_full[j * IN_F:(j + 1) * IN_F] = D[:, :, j].T.astype(np.float32)

    w_halves = []
    bias_halves = []
    for h in range(OUT_HALVES):
        wh = w_full[:, h * O_SHARD:(h + 1) * O_SHARD]
        # SBUF layout [128, N_KT, O_SHARD]: [p, kt, n] = wh[kt*128 + p, n]
        wh = np.ascontiguousarray(
            wh.reshape(N_KT, 128, O_SHARD).transpose(1, 0, 2))
        w_halves.append(wh)
        bias_halves.append(np.ascontiguousarray(
            bias_eff[h * O_SHARD:(h + 1) * O_SHARD].astype(np.float32)[None, :]))

    xt_groups = []
    for g in range(BATCH_GROUPS):
        xs = x[g * B_SHARD:(g + 1) * B_SHARD]              # (B_SHARD, IN_F)
        # [it, p, b] = xs[b, it*128 + p]
        xt = np.ascontiguousarray(xs.T.reshape(IT_PER_BLOCK, 128, B_SHARD))
        xt_groups.append(xt)
    return xt_groups, w_halves, bias_halves


def kernel(x, base_weight, jacobi_coeffs, bias):
    x = np.asarray(x, dtype=np.float32)
    base_weight = np.asarray(base_weight, dtype=np.float32)
    jacobi_coeffs = np.asarray(jacobi_coeffs, dtype=np.float32)
    bias = np.asarray(bias, dtype=np.float32)

    xt_groups, w_halves, bias_halves = _prepare_host(
        x, base_weight, jacobi_coeffs, bias)

    in_maps = []
    for c in range(N_CORES):
        g, h = c // OUT_HALVES, c % OUT_HALVES
        in_maps.append({
            "xt": xt_groups[g],
            "w": w_halves[h],
            "biasv": bias_halves[h],
            "onesv": np.ones((1, 128), dtype=np.float32),
        })

    nc = _get_nc()
    res = run_bass_kernel_spmd(nc, in_maps, core_ids=list(range(N_CORES)))

    out = np.empty((BATCH, OUT_F), dtype=np.float32)
    for c in range(N_CORES):
        g, h = c // OUT_HALVES, c % OUT_HALVES
        out[g * B_SHARD:(g + 1) * B_SHARD,
            h * O_SHARD:(h + 1) * O_SHARD] = res.results[c]["out"]
    return out



# revision 2
# speedup vs baseline: 1.1756x; 1.1756x over previous
"""JacobiKANLinear TRN2 Bass kernel.

out = silu(x) @ W_base^T + einsum('bik,oik->bo', P(tanh(x)), C) + bias

Host-side algebra: Jacobi polynomials (A=B=1, degree 5) are re-expressed in
the monomial basis.  D[o,i,j] = sum_k C[o,i,k] * T[k,j] where T holds the
monomial coefficients of P_k.  The j=0 term is constant (t^0 == 1) and folds
into the bias.  The device then computes 6 feature blocks
[silu(x), t, t^2, t^3, t^4, t^5] (t = tanh(x)) and one fused matmul with
contraction 6*1024 = 6144.

Sharding (8 cores): 4 batch groups x 2 out-feature halves.  Per core:
batch shard 2048 rows, out shard 512 cols.  Matmuls run in float32r
(1 cycle/row on the PE at N=512).

Schedule: the 12.6 MB weight block takes ~30 us to stream from HBM, so the
first P1 batch chunks are processed kt-major (all P1 chunks consume each
weight tile as it lands), which keeps the PE busy through the weight load.
Remaining chunks run chunk-major.  Bias is folded into the PSUM->SBUF copy
via a broadcast tile built once with a warm-up matmul (which also starts
the PE p-state ramp early); the Silu activation table is preloaded at t=0.
"""
import numpy as np

import concourse.bass as bass
import concourse.mybir as mybir
import concourse.tile as tile
from concourse import bacc
from concourse.bass_utils import run_bass_kernel_spmd

BATCH = 8192
IN_F = 1024
OUT_F = 1024
DEGREE = 5
A = 1.0
B = 1.0

N_CORES = 8
BATCH_GROUPS = 4
OUT_HALVES = 2
B_SHARD = BATCH // BATCH_GROUPS        # 2048
O_SHARD = OUT_F // OUT_HALVES          # 512
N_BLOCKS = DEGREE + 1                  # 6 feature blocks
N_KT = N_BLOCKS * IN_F // 128          # 48 contraction tiles of 128
N_CHUNKS = B_SHARD // 128              # 16 batch chunks per core
IT_PER_BLOCK = IN_F // 128             # 8 in-feature tiles per block

P1 = 4                                 # chunks processed kt-major during w load
SETS = 5                               # rotating feature-block tag sets

F32 = mybir.dt.float32
F32R = mybir.dt.float32r
SILU = mybir.ActivationFunctionType.Silu
TANH = mybir.ActivationFunctionType.Tanh


def _jacobi_monomial_matrix():
    """T[k, j] = coefficient of t^j in P_k (A=B=1), float64."""
    T = np.zeros((DEGREE + 1, DEGREE + 1), dtype=np.float64)
    polys = [np.zeros(DEGREE + 1) for _ in range(DEGREE + 1)]
    polys[0][0] = 1.0
    if DEGREE >= 1:
        polys[1][1] = A + 1.0
        polys[1][0] = 0.5 * (A - B)
    for k in range(2, DEGREE + 1):
        alpha_n = 2.0 * k * (k + A + B) * (2 * k + A + B - 2)
        beta_n = (2 * k + A + B - 1) * (A ** 2 - B ** 2)
        gamma_n = (2 * k + A + B - 2) * (2 * k + A + B - 1) * (2 * k + A + B)
        delta_n = 2.0 * (k + A - 1) * (k + B - 1) * (2 * k + A + B)
        p = np.zeros(DEGREE + 1)
        p += (beta_n / gamma_n) * polys[k - 1]
        p[1:] += (alpha_n / gamma_n) * polys[k - 1][:-1]
        p -= (delta_n / gamma_n) * polys[k - 2]
        polys[k] = p
    for k in range(DEGREE + 1):
        T[k] = polys[k]
    return T


def _build_nc():
    nc = bacc.Bacc()
    xt_in = nc.declare_dram_parameter(
        "xt", [N_CHUNKS, 128, IT_PER_BLOCK, 128], F32, isOutput=False)
    w_in = nc.declare_dram_parameter(
        "w", [128, N_KT, O_SHARD], F32R, isOutput=False)
    bias_in = nc.declare_dram_parameter("biasv", [1, O_SHARD], F32R, isOutput=False)
    ones_in = nc.declare_dram_parameter("onesv", [1, 128], F32R, isOutput=False)
    out = nc.declare_dram_parameter("out", [B_SHARD, O_SHARD], F32, isOutput=True)

    with tile.TileContext(nc) as tc:
        with tc.tile_pool(name="wpool", bufs=1) as wpool, \
             tc.tile_pool(name="bpool", bufs=1) as bpool, \
             tc.tile_pool(name="xpool", bufs=4) as xpool, \
             tc.tile_pool(name="opool", bufs=3) as opool, \
             tc.tile_pool(name="psum", bufs=1, space="PSUM") as psum_pool:

            w_sb = wpool.tile([128, N_KT, O_SHARD], F32R)
            bias_sb = wpool.tile([1, O_SHARD], F32R)
            ones_sb = wpool.tile([1, 128], F32R)
            bias_bc = wpool.tile([128, O_SHARD], F32)
            warm_a = wpool.tile([1, 8], F32)
            warm_b = wpool.tile([1, 8], F32)

            # Act stream: x0 DMA first, then the table preload runs during
            # the x0 transfer, then the remaining phase-1 x DMAs.
            x_tiles = []
            x_0 = xpool.tile([128, IT_PER_BLOCK, 128], F32, tag="x", name="x_0")
            nc.scalar.dma_start(out=x_0[:], in_=xt_in[0])
            x_tiles.append(x_0)
            nc.gpsimd.memset(warm_a[:], 0.0)
            nc.scalar.activation(warm_b[:], warm_a[:], SILU)
            for c in range(1, P1):
                x_c = xpool.tile([128, IT_PER_BLOCK, 128], F32, tag="x",
                                 name=f"x_{c}")
                nc.scalar.dma_start(out=x_c[:], in_=xt_in[c])
                x_tiles.append(x_c)

            # SP stream: bias/ones, then the 48 weight tiles in kt order.
            nc.sync.dma_start(out=bias_sb[:], in_=bias_in[:])
            nc.sync.dma_start(out=ones_sb[:], in_=ones_in[:])
            for kt in range(N_KT):
                nc.sync.dma_start(out=w_sb[:, kt, :], in_=w_in[:, kt, :])

            # PE warm-up: bias broadcast build doubles as p-state ramp start.
            warm_ps = psum_pool.tile([128, O_SHARD], F32, tag="warm")
            for _ in range(2):
                nc.tensor.matmul(
                    warm_ps[:], ones_sb[:], bias_sb[:], start=True, stop=True)
            nc.vector.tensor_copy(bias_bc[:], warm_ps[:])

            def make_blocks(s, x_c):
                a_t = bpool.tile([128, IT_PER_BLOCK, 128], F32R,
                                 tag=f"A{s}", name=f"blkA{s}")
                b_t = bpool.tile([128, IT_PER_BLOCK, 128], F32R,
                                 tag=f"B{s}", name=f"blkB{s}")
                t2 = bpool.tile([128, IT_PER_BLOCK, 128], F32R,
                                tag=f"C{s}", name=f"blkC{s}")
                t3 = bpool.tile([128, IT_PER_BLOCK, 128], F32R,
                                tag=f"D{s}", name=f"blkD{s}")
                nc.scalar.activation(a_t[:], x_c[:], SILU)
                nc.scalar.activation(b_t[:], x_c[:], TANH)
                nc.vector.tensor_mul(t2[:], b_t[:], b_t[:])
                nc.vector.tensor_mul(t3[:], t2[:], b_t[:])
                return a_t, b_t, t2, t3

            def emit_lategen(b, blks):
                # t4 reuses the silu slot, t5 the tanh slot; emitted between
                # their consumers so subtile deps order reads around writes.
                for a_t, b_t, t2, t3 in blks:
                    if b == 4:
                        nc.gpsimd.tensor_mul(a_t[:], t2[:], t2[:])
                    elif b == 5:
                        nc.gpsimd.tensor_mul(b_t[:], t2[:], t3[:])

            def finish_chunk(m, ps):
                o_m = opool.tile([128, O_SHARD], F32, tag="o", name=f"o_{m}")
                nc.vector.tensor_add(o_m[:], ps[:], bias_bc[:])
                nc.sync.dma_start(out=out[bass.ts(m, 128), :], in_=o_m[:])

            # Phase 1: chunks 0..P1-1, kt-major so every weight tile is
            # consumed P1 times as soon as it lands.
            blocks1 = [make_blocks(c, x_tiles[c]) for c in range(P1)]
            ps1 = [psum_pool.tile([128, O_SHARD], F32, tag="ps", bufs=P1 + 1,
                                  name=f"ps1_{c}") for c in range(P1)]
            for b in range(N_BLOCKS):
                if b >= 4:
                    emit_lategen(b, blocks1)
                for it in range(IT_PER_BLOCK):
                    kt = b * IT_PER_BLOCK + it
                    for c in range(P1):
                        a_t, b_t, t2, t3 = blocks1[c]
                        src = (a_t, b_t, t2, t3, a_t, b_t)[b]
                        nc.tensor.matmul(
                            ps1[c][:], src[:, it, :], w_sb[:, kt, :],
                            start=(kt == 0), stop=(kt == N_KT - 1))
            for c in range(P1):
                finish_chunk(c, ps1[c])

            # Phase 2: remaining chunks, chunk-major (weights resident).
            for m in range(P1, N_CHUNKS):
                x_m = xpool.tile([128, IT_PER_BLOCK, 128], F32, tag="x",
                                 name=f"x_{m}")
                nc.scalar.dma_start(out=x_m[:], in_=xt_in[m])
                blks = make_blocks(m % SETS, x_m)
                a_t, b_t, t2, t3 = blks
                ps = psum_pool.tile([128, O_SHARD], F32, tag="ps", bufs=P1 + 1,
                                    name=f"ps_{m}")
                for b in range(N_BLOCKS):
                    if b >= 4:
                        emit_lategen(b, [blks])
                    for it in range(IT_PER_BLOCK):
                        kt = b * IT_PER_BLOCK + it
                        src = (a_t, b_t, t2, t3, a_t, b_t)[b]
                        nc.tensor.matmul(
                            ps[:], src[:, it, :], w_sb[:, kt, :],
                            start=(kt == 0), stop=(kt == N_KT - 1))
                finish_chunk(m, ps)
    nc.finalize()
    return nc


_NC_CACHE = None


def _get_nc():
    global _NC_CACHE
    if _NC_CACHE is None:
        _NC_CACHE = _build_nc()
    return _NC_CACHE


def _prepare_host(x, base_weight, jacobi_coeffs, bias):
    T = _jacobi_monomial_matrix()
    D = np.einsum("oik,kj->oij", jacobi_coeffs.astype(np.float64), T)
    bias_eff = bias.astype(np.float64) + D[:, :, 0].sum(axis=1)

    # W'[f, o]: 6 blocks of IN_F feature rows: silu -> base_weight, t^j -> D_j
    w_full = np.empty((N_BLOCKS * IN_F, OUT_F), dtype=np.float32)
    w_full[0:IN_F] = base_weight.T
    for j in range(1, N_BLOCKS):
        w_full[j * IN_F:(j + 1) * IN_F] = D[:, :, j].T.astype(np.float32)

    w_halves = []
    bias_halves = []
    for h in range(OUT_HALVES):
        wh = w_full[:, h * O_SHARD:(h + 1) * O_SHARD]
        # SBUF layout [128, N_KT, O_SHARD]: [p, kt, n] = wh[kt*128 + p, n]
        wh = np.ascontiguousarray(
            wh.reshape(N_KT, 128, O_SHARD).transpose(1, 0, 2))
        w_halves.append(wh)
        bias_halves.append(np.ascontiguousarray(
            bias_eff[h * O_SHARD:(h + 1) * O_SHARD].astype(np.float32)[None, :]))

    xt_groups = []
    for g in range(BATCH_GROUPS):
        xs = x[g * B_SHARD:(g + 1) * B_SHARD]              # (B_SHARD, IN_F)
        # [c, p, it, b] = xs[c*128 + b, it*128 + p]
        xt = np.ascontiguousarray(
            xs.reshape(N_CHUNKS, 128, IT_PER_BLOCK, 128).transpose(0, 3, 2, 1))
        xt_groups.append(xt)
    return xt_groups, w_halves, bias_halves


def kernel(x, base_weight, jacobi_coeffs, bias):
    x = np.asarray(x, dtype=np.float32)
    base_weight = np.asarray(base_weight, dtype=np.float32)
    jacobi_coeffs = np.asarray(jacobi_coeffs, dtype=np.float32)
    bias = np.asarray(bias, dtype=np.float32)

    xt_groups, w_halves, bias_halves = _prepare_host(
        x, base_weight, jacobi_coeffs, bias)

    in_maps = []
    for c in range(N_CORES):
        g, h = c // OUT_HALVES, c % OUT_HALVES
        in_maps.append({
            "xt": xt_groups[g],
            "w": w_halves[h],
            "biasv": bias_halves[h],
            "onesv": np.ones((1, 128), dtype=np.float32),
        })

    nc = _get_nc()
    res = run_bass_kernel_spmd(nc, in_maps, core_ids=list(range(N_CORES)))

    out = np.empty((BATCH, OUT_F), dtype=np.float32)
    for c in range(N_CORES):
        g, h = c // OUT_HALVES, c % OUT_HALVES
        out[g * B_SHARD:(g + 1) * B_SHARD,
            h * O_SHARD:(h + 1) * O_SHARD] = res.results[c]["out"]
    return out


# revision 4
# speedup vs baseline: 1.1882x; 1.0107x over previous
"""JacobiKANLinear TRN2 Bass kernel.

out = silu(x) @ W_base^T + einsum('bik,oik->bo', P(tanh(x)), C) + bias

Host-side algebra: Jacobi polynomials (A=B=1, degree 5) are re-expressed in
the monomial basis.  D[o,i,j] = sum_k C[o,i,k] * T[k,j] where T holds the
monomial coefficients of P_k.  The j=0 term is constant (t^0 == 1) and folds
into the bias.  The device then computes 6 feature blocks
[silu(x), t, t^2, t^3, t^4, t^5] (t = tanh(x)) and one fused matmul with
contraction 6*1024 = 6144.

Sharding (8 cores): 4 batch groups x 2 out-feature halves.  Per core:
batch shard 2048 rows, out shard 512 cols.  Matmuls run in float32r
(1 cycle/row on the PE at N=512).

Schedule: the 12.6 MB weight block takes ~30 us to stream from HBM, so the
first P1 batch chunks are processed kt-major (all P1 chunks consume each
weight tile as it lands), which keeps the PE busy through the weight load.
Remaining chunks run chunk-major.  Bias is folded into the PSUM->SBUF copy
via a broadcast tile built once with a warm-up matmul (which also starts
the PE p-state ramp early); the Silu activation table is preloaded at t=0.
"""
import numpy as np

import concourse.bass as bass
import concourse.mybir as mybir
import concourse.tile as tile
from concourse import bacc
from concourse.bass_utils import run_bass_kernel_spmd

BATCH = 8192
IN_F = 1024
OUT_F = 1024
DEGREE = 5
A = 1.0
B = 1.0

N_CORES = 8
BATCH_GROUPS = 4
OUT_HALVES = 2
B_SHARD = BATCH // BATCH_GROUPS        # 2048
O_SHARD = OUT_F // OUT_HALVES          # 512
N_BLOCKS = DEGREE + 1                  # 6 feature blocks
N_KT = N_BLOCKS * IN_F // 128          # 48 contraction tiles of 128
N_CHUNKS = B_SHARD // 128              # 16 batch chunks per core
IT_PER_BLOCK = IN_F // 128             # 8 in-feature tiles per block

P1 = 4                                 # chunks processed kt-major during w load
SETS = 5                               # rotating feature-block tag sets

F32 = mybir.dt.float32
F32R = mybir.dt.float32r
SILU = mybir.ActivationFunctionType.Silu
TANH = mybir.ActivationFunctionType.Tanh


def _jacobi_monomial_matrix():
    """T[k, j] = coefficient of t^j in P_k (A=B=1), float64."""
    T = np.zeros((DEGREE + 1, DEGREE + 1), dtype=np.float64)
    polys = [np.zeros(DEGREE + 1) for _ in range(DEGREE + 1)]
    polys[0][0] = 1.0
    if DEGREE >= 1:
        polys[1][1] = A + 1.0
        polys[1][0] = 0.5 * (A - B)
    for k in range(2, DEGREE + 1):
        alpha_n = 2.0 * k * (k + A + B) * (2 * k + A + B - 2)
        beta_n = (2 * k + A + B - 1) * (A ** 2 - B ** 2)
        gamma_n = (2 * k + A + B - 2) * (2 * k + A + B - 1) * (2 * k + A + B)
        delta_n = 2.0 * (k + A - 1) * (k + B - 1) * (2 * k + A + B)
        p = np.zeros(DEGREE + 1)
        p += (beta_n / gamma_n) * polys[k - 1]
        p[1:] += (alpha_n / gamma_n) * polys[k - 1][:-1]
        p -= (delta_n / gamma_n) * polys[k - 2]
        polys[k] = p
    for k in range(DEGREE + 1):
        T[k] = polys[k]
    return T


def _build_nc():
    nc = bacc.Bacc()
    xt_in = nc.declare_dram_parameter(
        "xt", [N_CHUNKS, 128, IT_PER_BLOCK, 128], F32, isOutput=False)
    w_in = nc.declare_dram_parameter(
        "w", [128, N_KT, O_SHARD], F32R, isOutput=False)
    bias_in = nc.declare_dram_parameter("biasv", [1, O_SHARD], F32R, isOutput=False)
    ones_in = nc.declare_dram_parameter("onesv", [1, 128], F32R, isOutput=False)
    out = nc.declare_dram_parameter("out", [B_SHARD, O_SHARD], F32, isOutput=True)

    with tile.TileContext(nc) as tc:
        with tc.tile_pool(name="wpool", bufs=1) as wpool, \
             tc.tile_pool(name="bpool", bufs=1) as bpool, \
             tc.tile_pool(name="xpool", bufs=4) as xpool, \
             tc.tile_pool(name="opool", bufs=3) as opool, \
             tc.tile_pool(name="psum", bufs=1, space="PSUM") as psum_pool:

            w_sb = wpool.tile([128, N_KT, O_SHARD], F32R)
            bias_sb = wpool.tile([1, O_SHARD], F32R)
            ones_sb = wpool.tile([1, 128], F32R)
            bias_bc = wpool.tile([128, O_SHARD], F32)

            # Act stream: x0 DMA only (activations follow); the remaining
            # phase-1 x DMAs interleave with the first weight tiles on SP so
            # neither the weight stream nor silu_c is starved.
            x_tiles = []
            x_0 = xpool.tile([128, IT_PER_BLOCK, 128], F32, tag="x", name="x_0")
            nc.scalar.dma_start(out=x_0[:], in_=xt_in[0])
            x_tiles.append(x_0)

            # SP stream: bias/ones, then weights in kt order with the
            # phase-1 x tiles interleaved among the first few.
            nc.sync.dma_start(out=bias_sb[:], in_=bias_in[:])
            nc.sync.dma_start(out=ones_sb[:], in_=ones_in[:])
            for kt in range(N_KT):
                nc.sync.dma_start(out=w_sb[:, kt, :], in_=w_in[:, kt, :])
                if 1 <= kt < P1:
                    x_c = xpool.tile([128, IT_PER_BLOCK, 128], F32, tag="x",
                                     name=f"x_{kt}")
                    nc.sync.dma_start(out=x_c[:], in_=xt_in[kt])
                    x_tiles.append(x_c)

            # PE warm-up: bias broadcast build doubles as p-state ramp start.
            warm_ps = psum_pool.tile([128, O_SHARD], F32, tag="warm")
            for _ in range(2):
                nc.tensor.matmul(
                    warm_ps[:], ones_sb[:], bias_sb[:], start=True, stop=True)
            nc.vector.tensor_copy(bias_bc[:], warm_ps[:])

            def make_blocks(s, x_c):
                a_t = bpool.tile([128, IT_PER_BLOCK, 128], F32R,
                                 tag=f"A{s}", name=f"blkA{s}")
                b_t = bpool.tile([128, IT_PER_BLOCK, 128], F32R,
                                 tag=f"B{s}", name=f"blkB{s}")
                t2 = bpool.tile([128, IT_PER_BLOCK, 128], F32R,
                                tag=f"C{s}", name=f"blkC{s}")
                t3 = bpool.tile([128, IT_PER_BLOCK, 128], F32R,
                                tag=f"D{s}", name=f"blkD{s}")
                nc.scalar.activation(a_t[:], x_c[:], SILU)
                nc.scalar.activation(b_t[:], x_c[:], TANH)
                nc.vector.tensor_mul(t2[:], b_t[:], b_t[:])
                nc.vector.tensor_mul(t3[:], t2[:], b_t[:])
                return a_t, b_t, t2, t3

            def emit_lategen(b, blks):
                # t4 reuses the silu slot, t5 the tanh slot; emitted between
                # their consumers so subtile deps order reads around writes.
                for a_t, b_t, t2, t3 in blks:
                    if b == 4:
                        nc.gpsimd.tensor_mul(a_t[:], t2[:], t2[:])
                    elif b == 5:
                        nc.gpsimd.tensor_mul(b_t[:], t2[:], t3[:])

            def finish_chunk(m, ps):
                o_m = opool.tile([128, O_SHARD], F32, tag="o", name=f"o_{m}")
                nc.vector.tensor_add(o_m[:], ps[:], bias_bc[:])
                nc.sync.dma_start(out=out[bass.ts(m, 128), :], in_=o_m[:])

            # Phase 1: chunks 0..P1-1, kt-major so every weight tile is
            # consumed P1 times as soon as it lands.  Emit all silus before
            # the later tanhs so the kt=0..7 chunk interleave is not gated
            # on the Act engine working through tanh_c.
            blocks1 = []
            for c in range(P1):
                a_t = bpool.tile([128, IT_PER_BLOCK, 128], F32R,
                                 tag=f"A{c}", name=f"blkA{c}")
                b_t = bpool.tile([128, IT_PER_BLOCK, 128], F32R,
                                 tag=f"B{c}", name=f"blkB{c}")
                t2 = bpool.tile([128, IT_PER_BLOCK, 128], F32R,
                                tag=f"C{c}", name=f"blkC{c}")
                t3 = bpool.tile([128, IT_PER_BLOCK, 128], F32R,
                                tag=f"D{c}", name=f"blkD{c}")
                blocks1.append((a_t, b_t, t2, t3))
            nc.scalar.activation(blocks1[0][0][:], x_tiles[0][:], SILU)
            nc.scalar.activation(blocks1[0][1][:], x_tiles[0][:], TANH)
            for c in range(1, P1):
                nc.scalar.activation(blocks1[c][0][:], x_tiles[c][:], SILU)
            for c in range(1, P1):
                nc.scalar.activation(blocks1[c][1][:], x_tiles[c][:], TANH)
            for c in range(P1):
                a_t, b_t, t2, t3 = blocks1[c]
                nc.vector.tensor_mul(t2[:], b_t[:], b_t[:])
                nc.vector.tensor_mul(t3[:], t2[:], b_t[:])
            ps1 = [psum_pool.tile([128, O_SHARD], F32, tag="ps", bufs=P1 + 1,
                                  name=f"ps1_{c}") for c in range(P1)]
            for b in range(N_BLOCKS):
                if b >= 4:
                    emit_lategen(b, blocks1)
                for it in range(IT_PER_BLOCK):
                    kt = b * IT_PER_BLOCK + it
                    for c in range(P1):
                        a_t, b_t, t2, t3 = blocks1[c]
                        src = (a_t, b_t, t2, t3, a_t, b_t)[b]
                        nc.tensor.matmul(
                            ps1[c][:], src[:, it, :], w_sb[:, kt, :],
                            start=(kt == 0), stop=(kt == N_KT - 1))
            for c in range(P1):
                finish_chunk(c, ps1[c])

            # Phase 2: remaining chunks, chunk-major (weights resident).
            for m in range(P1, N_CHUNKS):
                x_m = xpool.tile([128, IT_PER_BLOCK, 128], F32, tag="x",
                                 name=f"x_{m}")
                nc.scalar.dma_start(out=x_m[:], in_=xt_in[m])
                blks = make_blocks(m % SETS, x_m)
                a_t, b_t, t2, t3 = blks
                ps = psum_pool.tile([128, O_SHARD], F32, tag="ps", bufs=P1 + 1,
                                    name=f"ps_{m}")
                for b in range(N_BLOCKS):
                    if b >= 4:
                        emit_lategen(b, [blks])
                    for it in range(IT_PER_BLOCK):
                        kt = b * IT_PER_BLOCK + it
                        src = (a_t, b_t, t2, t3, a_t, b_t)[b]
                        nc.tensor.matmul(
                            ps[:], src[:, it, :], w_sb[:, kt, :],
                            start=(kt == 0), stop=(kt == N_KT - 1))
                finish_chunk(m, ps)
    nc.finalize()
    return nc


_NC_CACHE = None


def _get_nc():
    global _NC_CACHE
    if _NC_CACHE is None:
        _NC_CACHE = _build_nc()
    return _NC_CACHE


def _prepare_host(x, base_weight, jacobi_coeffs, bias):
    T = _jacobi_monomial_matrix()
    D = np.einsum("oik,kj->oij", jacobi_coeffs.astype(np.float64), T)
    bias_eff = bias.astype(np.float64) + D[:, :, 0].sum(axis=1)

    # W'[f, o]: 6 blocks of IN_F feature rows: silu -> base_weight, t^j -> D_j
    w_full = np.empty((N_BLOCKS * IN_F, OUT_F), dtype=np.float32)
    w_full[0:IN_F] = base_weight.T
    for j in range(1, N_BLOCKS):
        w_full[j * IN_F:(j + 1) * IN_F] = D[:, :, j].T.astype(np.float32)

    w_halves = []
    bias_halves = []
    for h in range(OUT_HALVES):
        wh = w_full[:, h * O_SHARD:(h + 1) * O_SHARD]
        # SBUF layout [128, N_KT, O_SHARD]: [p, kt, n] = wh[kt*128 + p, n]
        wh = np.ascontiguousarray(
            wh.reshape(N_KT, 128, O_SHARD).transpose(1, 0, 2))
        w_halves.append(wh)
        bias_halves.append(np.ascontiguousarray(
            bias_eff[h * O_SHARD:(h + 1) * O_SHARD].astype(np.float32)[None, :]))

    xt_groups = []
    for g in range(BATCH_GROUPS):
        xs = x[g * B_SHARD:(g + 1) * B_SHARD]              # (B_SHARD, IN_F)
        # [c, p, it, b] = xs[c*128 + b, it*128 + p]
        xt = np.ascontiguousarray(
            xs.reshape(N_CHUNKS, 128, IT_PER_BLOCK, 128).transpose(0, 3, 2, 1))
        xt_groups.append(xt)
    return xt_groups, w_halves, bias_halves


def kernel(x, base_weight, jacobi_coeffs, bias):
    x = np.asarray(x, dtype=np.float32)
    base_weight = np.asarray(base_weight, dtype=np.float32)
    jacobi_coeffs = np.asarray(jacobi_coeffs, dtype=np.float32)
    bias = np.asarray(bias, dtype=np.float32)

    xt_groups, w_halves, bias_halves = _prepare_host(
        x, base_weight, jacobi_coeffs, bias)

    in_maps = []
    for c in range(N_CORES):
        g, h = c // OUT_HALVES, c % OUT_HALVES
        in_maps.append({
            "xt": xt_groups[g],
            "w": w_halves[h],
            "biasv": bias_halves[h],
            "onesv": np.ones((1, 128), dtype=np.float32),
        })

    nc = _get_nc()
    res = run_bass_kernel_spmd(nc, in_maps, core_ids=list(range(N_CORES)))

    out = np.empty((BATCH, OUT_F), dtype=np.float32)
    for c in range(N_CORES):
        g, h = c // OUT_HALVES, c % OUT_HALVES
        out[g * B_SHARD:(g + 1) * B_SHARD,
            h * O_SHARD:(h + 1) * O_SHARD] = res.results[c]["out"]
    return out
